# revision 1
# baseline (speedup 1.0000x reference)
"""Trainium2 Bass kernel for single-head MultiHeadAttention (b=8, n=2048, d=e=512).

Sharding: data-parallel over batch b across 8 NeuronCores (core i handles
batch element i). No collectives needed.

Per-core math (one batch element, all matmuls in bf16, fp32 PSUM accum):
  qpT[e,n] = sum_d Wq[e,d] q[n,d]        (raw, no scale; T-layout for dots)
  kpT[e,m] = sum_d Wk[e,d] k[m,d]
  vp [m,e] = sum_d v[m,d]  Wv[e,d]       (raw, natural layout for PV)
  dots_raw[n,m] = sum_e qpT[e,n] kpT[e,m]
  expu = exp(dots_raw * s)   with s = 512**-1.5  (folds both 1/sqrt(512)
         EqualizedLinear scales and the attention scale into one scalar;
         |dots| <~ 6 so softmax needs no max subtraction)
  rowsum[n] = sum_m expu[n,m]
  attn = expu / rowsum                    (fp32 output)
  out[n,e] = (sum_m expu[n,m] vp[m,e]) * (1/(rowsum[n]*sqrt(512)))

Transposes (q/k/v [n,d]->[d,n] and expu [n,m]->[m,n]) run on the DMA
transpose crossbar (bf16, 128x128 blocks), keeping the PE array free for
matmuls.
"""

import sys

sys.path.insert(0, "/opt/trn_rl_repo")

import numpy as np

import concourse.bass as bass
import concourse.tile as tile
from concourse import bacc, mybir

P = 128
N = 2048  # sequence length per batch element
D = 512   # input dim
E = 512   # embed dim
B = 8     # batch == number of cores

DT = D // P   # 4 k-tiles over d
ET = E // P   # 4 e-chunks
MT = N // P   # 16 m-tiles
NCH = N // P  # 16 n-chunks of 128 rows
NSC = 4       # super-chunks of 512 rows for input staging

S_EXP = float(D) ** -1.5    # folded scale inside exp
C_V = float(D) ** -0.5      # EqualizedLinear scale for the v projection

F32 = mybir.dt.float32
BF16 = mybir.dt.bfloat16
AF = mybir.ActivationFunctionType


def build_nc():
    nc = bacc.Bacc("TRN2", target_bir_lowering=False, debug=False)

    q_d = nc.dram_tensor("q", [N, D], F32, kind="ExternalInput")
    k_d = nc.dram_tensor("k", [N, D], F32, kind="ExternalInput")
    v_d = nc.dram_tensor("v", [N, D], F32, kind="ExternalInput")
    wq_d = nc.dram_tensor("Wq", [E, D], F32, kind="ExternalInput")
    wk_d = nc.dram_tensor("Wk", [E, D], F32, kind="ExternalInput")
    wv_d = nc.dram_tensor("Wv", [E, D], F32, kind="ExternalInput")
    attn_d = nc.dram_tensor("attn", [N, N], F32, kind="ExternalOutput")
    out_d = nc.dram_tensor("out", [N, E], F32, kind="ExternalOutput")

    with tile.TileContext(nc) as tc:
        with (
            tc.tile_pool(name="consts", bufs=1) as consts,
            tc.tile_pool(name="proj", bufs=1) as proj,
            tc.tile_pool(name="stage", bufs=3) as stage,
            tc.tile_pool(name="stage_bf", bufs=3) as stage_bf,
            tc.tile_pool(name="stage_t", bufs=3) as stage_t,
            tc.tile_pool(name="p2_exp", bufs=3) as p2_exp,
            tc.tile_pool(name="p2_attn", bufs=2) as p2_attn,
            tc.tile_pool(name="p2_expt", bufs=2) as p2_expt,
            tc.tile_pool(name="p2_out", bufs=2) as p2_out,
            tc.tile_pool(name="small", bufs=8) as small,
            tc.tile_pool(name="ps_proj", bufs=2, space="PSUM") as ps_proj,
            tc.tile_pool(name="ps_dots", bufs=3, space="PSUM") as ps_dots,
            tc.tile_pool(name="ps_pv", bufs=2, space="PSUM") as ps_pv,
        ):
            # ---- weights: load fp32, cast bf16, xbar-transpose to [d, e] ----
            wts = {}
            for name, w_d in (("wq", wq_d), ("wk", wk_d), ("wv", wv_d)):
                w_f32 = stage.tile([P, ET, D], F32, tag="ldstage")
                nc.sync.dma_start(
                    out=w_f32, in_=w_d.rearrange("(et p) d -> p et d", p=P)
                )
                w_bf = stage_bf.tile([P, ET, D], BF16, tag="bfstage")
                nc.vector.tensor_copy(w_bf, w_f32)
                wT = consts.tile([P, DT, E], BF16, tag=f"{name}T")
                for dt in range(DT):
                    for et in range(ET):
                        nc.sync.dma_start(
                            out=wT[:, dt, et * P:(et + 1) * P],
                            in_=w_bf[:, et, dt * P:(dt + 1) * P],
                            transpose=True,
                        )
                wts[name] = wT
            wqT, wkT, wvT = wts["wq"], wts["wk"], wts["wv"]

            # ---- projections ----
            # kpT/qpT resident [e_in, et, n] bf16 ; vp resident [m_in, mt, e]
            qpT = proj.tile([P, ET, N], BF16, tag="qpT")
            kpT = proj.tile([P, ET, N], BF16, tag="kpT")
            vp = proj.tile([P, MT, E], BF16, tag="vp")

            def load_and_transpose(x_d, sc):
                """Load 512 rows of x, cast to bf16, xbar-transpose.

                Returns xT [d_in, dt, n_local] bf16 (n_local = 512 rows of
                this super-chunk)."""
                x_f32 = stage.tile([P, 4, D], F32, tag="ldstage")
                nc.sync.dma_start(
                    out=x_f32,
                    in_=x_d[sc * 512:(sc + 1) * 512, :].rearrange(
                        "(c p) d -> p c d", p=P
                    ),
                )
                x_bf = stage_bf.tile([P, 4, D], BF16, tag="bfstage")
                nc.vector.tensor_copy(x_bf, x_f32)
                xT = stage_t.tile([P, DT, 512], BF16, tag="xT")
                for dt in range(DT):
                    for c in range(4):
                        nc.sync.dma_start(
                            out=xT[:, dt, c * P:(c + 1) * P],
                            in_=x_bf[:, c, dt * P:(dt + 1) * P],
                            transpose=True,
                        )
                return xT

            # k and q projections -> T layout [e, n]
            for name, x_d, dstT in (("k", k_d, kpT), ("q", q_d, qpT)):
                for sc in range(NSC):
                    xT = load_and_transpose(x_d, sc)
                    for et in range(ET):
                        ps = ps_proj.tile([P, 512], F32, tag="ps_proj")
                        wT = wkT if name == "k" else wqT
                        for dt in range(DT):
                            nc.tensor.matmul(
                                ps,
                                lhsT=wT[:, dt, et * P:(et + 1) * P],
                                rhs=xT[:, dt, :],
                                start=(dt == 0),
                                stop=(dt == DT - 1),
                            )
                        nc.vector.tensor_copy(
                            dstT[:, et, sc * 512:(sc + 1) * 512], ps
                        )

            # v projection -> natural layout [m, e]
            for sc in range(NSC):
                vT = load_and_transpose(v_d, sc)
                for mc in range(4):
                    mt = sc * 4 + mc
                    ps = ps_proj.tile([P, E], F32, tag="ps_proj")
                    for dt in range(DT):
                        nc.tensor.matmul(
                            ps,
                            lhsT=vT[:, dt, mc * P:(mc + 1) * P],
                            rhs=wvT[:, dt, :],
                            start=(dt == 0),
                            stop=(dt == DT - 1),
                        )
                    nc.vector.tensor_copy(vp[:, mt, :], ps)

            # ---- attention, one 128-row chunk at a time ----
            for ch in range(NCH):
                nsl = slice(ch * P, (ch + 1) * P)
                exp_bf = p2_exp.tile([P, N], BF16, tag="exp")
                rs4 = small.tile([P, 4], F32, tag="rs4")
                for mc in range(4):
                    ps = ps_dots.tile([P, 512], F32, tag="ps_dots")
                    for et in range(ET):
                        nc.tensor.matmul(
                            ps,
                            lhsT=qpT[:, et, nsl],
                            rhs=kpT[:, et, mc * 512:(mc + 1) * 512],
                            start=(et == 0),
                            stop=(et == ET - 1),
                        )
                    nc.scalar.activation(
                        out=exp_bf[:, mc * 512:(mc + 1) * 512],
                        in_=ps,
                        func=AF.Exp,
                        scale=S_EXP,
                        accum_out=rs4[:, mc:mc + 1],
                    )
                rowsum = small.tile([P, 1], F32, tag="rowsum")
                nc.vector.reduce_sum(rowsum, rs4, axis=mybir.AxisListType.X)
                rinv = small.tile([P, 1], F32, tag="rinv")
                nc.vector.reciprocal(rinv, rowsum)
                rinv_c = small.tile([P, 1], F32, tag="rinv_c")
                nc.scalar.mul(rinv_c, rinv, C_V)

                # normalized fp32 attention out to DRAM
                attn_f = p2_attn.tile([P, N], F32, tag="attn_f")
                nc.vector.tensor_scalar_mul(attn_f, in0=exp_bf, scalar1=rinv)
                nc.sync.dma_start(out=attn_d[nsl, :], in_=attn_f)

                # transpose unnormalized exp for the PV matmul
                expT = p2_expt.tile([P, MT, P], BF16, tag="expT")
                for mt in range(MT):
                    nc.sync.dma_start(
                        out=expT[:, mt, :],
                        in_=exp_bf[:, mt * P:(mt + 1) * P],
                        transpose=True,
                    )
                pso = ps_pv.tile([P, E], F32, tag="ps_pv")
                for mt in range(MT):
                    nc.tensor.matmul(
                        pso,
                        lhsT=expT[:, mt, :],
                        rhs=vp[:, mt, :],
                        start=(mt == 0),
                        stop=(mt == MT - 1),
                    )
                out_sb = p2_out.tile([P, E], F32, tag="out_sb")
                nc.scalar.activation(
                    out=out_sb,
                    in_=pso,
                    func=AF.Identity,
                    bias=0.0,
                    scale=rinv_c[:, 0:1],
                )
                nc.sync.dma_start(out=out_d[nsl, :], in_=out_sb)

    nc.compile()
    return nc


_CACHE = {}


def _get_nc():
    if "nc" not in _CACHE:
        _CACHE["nc"] = build_nc()
    return _CACHE["nc"]


def _make_in_maps(q, k, v, Wq, Wk, Wv):
    def f32(x):
        return np.ascontiguousarray(np.asarray(x), dtype=np.float32)

    wq, wk, wv = f32(Wq), f32(Wk), f32(Wv)
    return [
        {
            "q": f32(q[i]),
            "k": f32(k[i]),
            "v": f32(v[i]),
            "Wq": wq,
            "Wk": wk,
            "Wv": wv,
        }
        for i in range(B)
    ]


def run(q, k, v, Wq, Wk, Wv, trace=False, **kwargs):
    """Run on hardware; returns ((out, (attn,)), BassKernelResults)."""
    from concourse.bass_utils import run_bass_kernel_spmd

    nc = _get_nc()
    in_maps = _make_in_maps(q, k, v, Wq, Wk, Wv)
    br = run_bass_kernel_spmd(nc, in_maps, list(range(B)), trace=trace, **kwargs)
    res = br.results
    out = np.stack([res[i]["out"] for i in range(B)])
    attn = np.stack([res[i]["attn"] for i in range(B)])
    return (out, (attn,)), br


def kernel(q, k, v, Wq, Wk, Wv):
    outputs, _ = run(q, k, v, Wq, Wk, Wv)
    return outputs


# revision 5
# speedup vs baseline: 3.8119x; 3.8119x over previous
"""Trainium2 Bass kernel for single-head MultiHeadAttention (b=8, n=2048, d=e=512).

Sharding: data-parallel over batch b across 8 NeuronCores (core i handles
batch element i). No collectives needed.

Per-core math (one batch element, matmuls in bf16, fp32 PSUM accum):
  qpT[e,n] = sum_d Wq[e,d] q[n,d]        (raw, no scale; T-layout for dots)
  kpT[e,m] = sum_d Wk[e,d] k[m,d]
  vp [m,e] = sum_d v[m,d]  Wv[e,d]       (raw, natural layout for PV)
  dots_raw[n,m] = sum_e qpT[e,n] kpT[e,m]
  expu = exp(dots_raw * s)   with s = 512**-1.5  (folds both 1/sqrt(512)
         EqualizedLinear scales and the attention scale into one scalar;
         |dots| <~ 6 so softmax needs no max subtraction)
  rowsum[n] = sum_m expu[n,m]            (ACT accum_out during exp eviction)
  attn = expu / rowsum                    (fp32 output)
  out[n,e] = (sum_m expu[n,m] vp[m,e]) * (1/(rowsum[n]*sqrt(512)))

All [n,d]->[d,n] / [n,m]->[m,n] transposes run on the PE in transpose mode
(128x128 blocks, 4 blocks grouped into one PSUM bank so the PSUM->SBUF
eviction is a single [128,512] copy that also performs the fp32->bf16
cast). The DMA xbar transpose is NOT used: it occupies the sync sequencer
~1.2us per 128x128 call (measured), which serialized the whole kernel.
"""

import sys

sys.path.insert(0, "/opt/trn_rl_repo")

import numpy as np

import concourse.bass as bass
import concourse.tile as tile
from concourse import bacc, mybir
from concourse.masks import make_identity

P = 128
N = 2048  # sequence length per batch element
D = 512   # input dim
E = 512   # embed dim
B = 8     # batch == number of cores

DT = D // P   # 4 k-tiles over d
ET = E // P   # 4 e-chunks
MT = N // P   # 16 m-tiles
NCH = N // P  # 16 n-chunks of 128 rows
NSC = 4       # super-chunks of 512 rows for input staging

S_EXP = float(D) ** -1.5    # folded scale inside exp
C_V = float(D) ** -0.5      # EqualizedLinear scale for the v projection

F32 = mybir.dt.float32
BF16 = mybir.dt.bfloat16
AF = mybir.ActivationFunctionType


def build_nc():
    nc = bacc.Bacc("TRN2", target_bir_lowering=False, debug=False)

    q_d = nc.dram_tensor("q", [N, D], F32, kind="ExternalInput")
    k_d = nc.dram_tensor("k", [N, D], F32, kind="ExternalInput")
    v_d = nc.dram_tensor("v", [N, D], F32, kind="ExternalInput")
    wq_d = nc.dram_tensor("Wq", [E, D], F32, kind="ExternalInput")
    wk_d = nc.dram_tensor("Wk", [E, D], F32, kind="ExternalInput")
    wv_d = nc.dram_tensor("Wv", [E, D], F32, kind="ExternalInput")
    attn_d = nc.dram_tensor("attn", [N, N], F32, kind="ExternalOutput")
    out_d = nc.dram_tensor("out", [N, E], F32, kind="ExternalOutput")

    with tile.TileContext(nc) as tc:
        with (
            tc.tile_pool(name="consts", bufs=1) as consts,
            tc.tile_pool(name="proj", bufs=1) as proj,
            tc.tile_pool(name="stage", bufs=3) as stage,
            tc.tile_pool(name="stage_t", bufs=3) as stage_t,
            tc.tile_pool(name="p2_exp", bufs=3) as p2_exp,
            tc.tile_pool(name="p2_attn", bufs=2) as p2_attn,
            tc.tile_pool(name="p2_expt", bufs=2) as p2_expt,
            tc.tile_pool(name="p2_out", bufs=2) as p2_out,
            tc.tile_pool(name="small", bufs=8) as small,
            tc.tile_pool(name="ps_proj", bufs=2, space="PSUM") as ps_proj,
            tc.tile_pool(name="ps_dots", bufs=2, space="PSUM") as ps_dots,
            tc.tile_pool(name="ps_pv", bufs=2, space="PSUM") as ps_pv,
            tc.tile_pool(name="ps_t", bufs=2, space="PSUM") as ps_t,
        ):
            id_f32 = consts.tile([P, P], F32, tag="id_f32")
            make_identity(nc, id_f32)
            id_bf16 = consts.tile([P, P], BF16, tag="id_bf16")
            make_identity(nc, id_bf16)

            # ---- weights: load fp32, PE-transpose to [d, e], evict as bf16 --
            wts = {}
            for name, w_d in (("wq", wq_d), ("wk", wk_d), ("wv", wv_d)):
                w_f32 = stage.tile([P, 4, D], F32, tag="ldstage")
                nc.sync.dma_start(
                    out=w_f32, in_=w_d.rearrange("(et p) d -> p et d", p=P)
                )
                wT = consts.tile([P, DT, E], BF16, tag=f"{name}T")
                for et in range(ET):
                    pst = ps_t.tile([P, 4 * P], F32, tag="ps_t")
                    for dt in range(DT):
                        nc.tensor.transpose(
                            pst[:, dt * P:(dt + 1) * P],
                            w_f32[:, et, dt * P:(dt + 1) * P],
                            id_f32,
                        )
                    # pst[d_in, dt-major] -> wT[:, :, et*128...] (cast bf16)
                    nc.vector.tensor_copy(
                        wT[:, :, et * P:(et + 1) * P],
                        pst.rearrange("p (a b) -> p a b", b=P),
                    )
                wts[name] = wT
            wqT, wkT, wvT = wts["wq"], wts["wk"], wts["wv"]

            # ---- projections ----
            # kpT/qpT resident [e_in, et, n] bf16 ; vp resident [m_in, mt, e]
            qpT = proj.tile([P, ET, N], BF16, tag="qpT")
            kpT = proj.tile([P, ET, N], BF16, tag="kpT")
            vp = proj.tile([P, MT, E], BF16, tag="vp")

            def load_and_transpose(x_d, sc, evict_engine):
                """Load 512 rows of x, PE-transpose into bf16 xT tile.

                Returns xT [d_in, dt, n_local] bf16 (n_local = 512 rows of
                this super-chunk)."""
                x_f32 = stage.tile([P, 4, D], F32, tag="ldstage")
                nc.sync.dma_start(
                    out=x_f32,
                    in_=x_d[sc * 512:(sc + 1) * 512, :].rearrange(
                        "(c p) d -> p c d", p=P
                    ),
                )
                xT = stage_t.tile([P, DT, 512], BF16, tag="xT")
                for c in range(4):
                    pst = ps_t.tile([P, 4 * P], F32, tag="ps_t")
                    for dt in range(DT):
                        nc.tensor.transpose(
                            pst[:, dt * P:(dt + 1) * P],
                            x_f32[:, c, dt * P:(dt + 1) * P],
                            id_f32,
                        )
                    evict_engine.tensor_copy(
                        xT[:, :, c * P:(c + 1) * P],
                        pst.rearrange("p (a b) -> p a b", b=P),
                    )
                return xT

            # k and q projections -> T layout [e, n]
            for wT, x_d, dstT in ((wkT, k_d, kpT), (wqT, q_d, qpT)):
                for sc in range(NSC):
                    xT = load_and_transpose(x_d, sc, nc.vector)
                    for et in range(ET):
                        ps = ps_proj.tile([P, 512], F32, tag="ps_proj")
                        for dt in range(DT):
                            nc.tensor.matmul(
                                ps,
                                lhsT=wT[:, dt, et * P:(et + 1) * P],
                                rhs=xT[:, dt, :],
                                start=(dt == 0),
                                stop=(dt == DT - 1),
                            )
                        nc.vector.tensor_copy(
                            dstT[:, et, sc * 512:(sc + 1) * 512], ps
                        )

            # v projection -> natural layout [m, e]
            for sc in range(NSC):
                vT = load_and_transpose(v_d, sc, nc.vector)
                for mc in range(4):
                    mt = sc * 4 + mc
                    ps = ps_proj.tile([P, E], F32, tag="ps_proj")
                    for dt in range(DT):
                        nc.tensor.matmul(
                            ps,
                            lhsT=vT[:, dt, mc * P:(mc + 1) * P],
                            rhs=wvT[:, dt, :],
                            start=(dt == 0),
                            stop=(dt == DT - 1),
                        )
                    nc.vector.tensor_copy(vp[:, mt, :], ps)

            # ---- attention, one 128-row chunk at a time ----
            for ch in range(NCH):
                nsl = slice(ch * P, (ch + 1) * P)
                exp_bf = p2_exp.tile([P, N], BF16, tag="exp")
                rs4 = small.tile([P, 4], F32, tag="rs4")
                for mc in range(4):
                    ps = ps_dots.tile([P, 512], F32, tag="ps_dots")
                    for et in range(ET):
                        nc.tensor.matmul(
                            ps,
                            lhsT=qpT[:, et, nsl],
                            rhs=kpT[:, et, mc * 512:(mc + 1) * 512],
                            start=(et == 0),
                            stop=(et == ET - 1),
                        )
                    nc.scalar.activation(
                        out=exp_bf[:, mc * 512:(mc + 1) * 512],
                        in_=ps,
                        func=AF.Exp,
                        scale=S_EXP,
                        accum_out=rs4[:, mc:mc + 1],
                    )
                rowsum = small.tile([P, 1], F32, tag="rowsum")
                nc.vector.reduce_sum(rowsum, rs4, axis=mybir.AxisListType.X)
                rinv = small.tile([P, 1], F32, tag="rinv")
                nc.vector.reciprocal(rinv, rowsum)
                rinv_c = small.tile([P, 1], F32, tag="rinv_c")
                nc.scalar.mul(rinv_c, rinv, C_V)

                # normalized fp32 attention out to DRAM
                attn_f = p2_attn.tile([P, N], F32, tag="attn_f")
                nc.vector.tensor_scalar_mul(attn_f, in0=exp_bf, scalar1=rinv)
                nc.sync.dma_start(out=attn_d[nsl, :], in_=attn_f)

                # PE-transpose unnormalized exp for the PV matmul
                expT = p2_expt.tile([P, MT, P], BF16, tag="expT")
                for g in range(4):
                    pst = ps_t.tile([P, 4 * P], BF16, tag="ps_t")
                    for j in range(4):
                        mt = 4 * g + j
                        nc.tensor.transpose(
                            pst[:, j * P:(j + 1) * P],
                            exp_bf[:, mt * P:(mt + 1) * P],
                            id_bf16,
                        )
                    src = pst.rearrange("p (a b) -> p a b", b=P)
                    dst = expT[:, 4 * g:4 * g + 4, :]
                    if g % 2 == 0:
                        nc.scalar.copy(dst, src)
                    else:
                        nc.vector.tensor_copy(dst, src)
                pso = ps_pv.tile([P, E], F32, tag="ps_pv")
                for mt in range(MT):
                    nc.tensor.matmul(
                        pso,
                        lhsT=expT[:, mt, :],
                        rhs=vp[:, mt, :],
                        start=(mt == 0),
                        stop=(mt == MT - 1),
                    )
                out_sb = p2_out.tile([P, E], F32, tag="out_sb")
                nc.scalar.activation(
                    out=out_sb,
                    in_=pso,
                    func=AF.Identity,
                    bias=0.0,
                    scale=rinv_c[:, 0:1],
                )
                nc.sync.dma_start(out=out_d[nsl, :], in_=out_sb)

    nc.compile()
    return nc


_CACHE = {}


def _get_nc():
    if "nc" not in _CACHE:
        _CACHE["nc"] = build_nc()
    return _CACHE["nc"]


def _make_in_maps(q, k, v, Wq, Wk, Wv):
    def f32(x):
        return np.ascontiguousarray(np.asarray(x), dtype=np.float32)

    wq, wk, wv = f32(Wq), f32(Wk), f32(Wv)
    return [
        {
            "q": f32(q[i]),
            "k": f32(k[i]),
            "v": f32(v[i]),
            "Wq": wq,
            "Wk": wk,
            "Wv": wv,
        }
        for i in range(B)
    ]


def run(q, k, v, Wq, Wk, Wv, trace=False, **kwargs):
    """Run on hardware; returns ((out, (attn,)), BassKernelResults)."""
    from concourse.bass_utils import run_bass_kernel_spmd

    nc = _get_nc()
    in_maps = _make_in_maps(q, k, v, Wq, Wk, Wv)
    br = run_bass_kernel_spmd(nc, in_maps, list(range(B)), trace=trace, **kwargs)
    res = br.results
    out = np.stack([res[i]["out"] for i in range(B)])
    attn = np.stack([res[i]["attn"] for i in range(B)])
    return (out, (attn,)), br


def kernel(q, k, v, Wq, Wk, Wv):
    outputs, _ = run(q, k, v, Wq, Wk, Wv)
    return outputs


# revision 11
# speedup vs baseline: 3.8650x; 1.0139x over previous
"""Trainium2 Bass kernel for single-head MultiHeadAttention (b=8, n=2048, d=e=512).

Sharding: data-parallel over batch b across 8 NeuronCores (core i handles
batch element i). No collectives needed.

Per-core math (one batch element, matmuls in bf16, fp32 PSUM accum):
  qpT[e,n] = sum_d Wq[e,d] q[n,d]        (raw, no scale; T-layout for dots)
  kpT[e,m] = sum_d Wk[e,d] k[m,d]
  vp [m,e] = sum_d v[m,d]  Wv[e,d]       (raw, natural layout for PV)
  dots_raw[n,m] = sum_e qpT[e,n] kpT[e,m]
  expu = exp(dots_raw * s)   with s = 512**-1.5  (folds both 1/sqrt(512)
         EqualizedLinear scales and the attention scale into one scalar;
         |dots| <~ 6 so softmax needs no max subtraction)
  rowsum[n] = sum_m expu[n,m]            (ACT accum_out during exp eviction)
  attn = expu / rowsum                    (fp32 output)
  out[n,e] = (sum_m expu[n,m] vp[m,e]) * (1/(rowsum[n]*sqrt(512)))

All [n,d]->[d,n] / [n,m]->[m,n] transposes run on the PE in transpose mode
on bf16 data (inputs are cast fp32->bf16 on DVE first, keeping the whole
PE instruction stream bf16). 128x128 transpose blocks are grouped 4-to-a
PSUM-bank so each PSUM->SBUF eviction is a single [128,512] copy.
Alternatives measured and rejected: DMA xbar transpose per 128x128 block
(~1.2us sync-sequencer occupancy per call -> serialized the kernel at
878us) and a bf16-DRAM-scratch + big-xbar-call round trip (stalls the PE
~50us waiting for staging -> 312us).
"""

import sys

sys.path.insert(0, "/opt/trn_rl_repo")

import numpy as np

import concourse.bass as bass
import concourse.tile as tile
from concourse import bacc, mybir
from concourse.masks import make_identity

P = 128
N = 2048  # sequence length per batch element
D = 512   # input dim
E = 512   # embed dim
B = 8     # batch == number of cores

DT = D // P   # 4 k-tiles over d
ET = E // P   # 4 e-chunks
MT = N // P   # 16 m-tiles
NCH = N // P  # 16 n-chunks of 128 rows
NSC = 4       # super-chunks of 512 rows for input staging

S_EXP = float(D) ** -1.5    # folded scale inside exp
C_V = float(D) ** -0.5      # EqualizedLinear scale for the v projection

F32 = mybir.dt.float32
BF16 = mybir.dt.bfloat16
AF = mybir.ActivationFunctionType


def build_nc():
    nc = bacc.Bacc("TRN2", target_bir_lowering=False, debug=False)

    q_d = nc.dram_tensor("q", [N, D], F32, kind="ExternalInput")
    k_d = nc.dram_tensor("k", [N, D], F32, kind="ExternalInput")
    v_d = nc.dram_tensor("v", [N, D], F32, kind="ExternalInput")
    wq_d = nc.dram_tensor("Wq", [E, D], F32, kind="ExternalInput")
    wk_d = nc.dram_tensor("Wk", [E, D], F32, kind="ExternalInput")
    wv_d = nc.dram_tensor("Wv", [E, D], F32, kind="ExternalInput")
    attn_d = nc.dram_tensor("attn", [N, N], F32, kind="ExternalOutput")
    out_d = nc.dram_tensor("out", [N, E], F32, kind="ExternalOutput")

    with tile.TileContext(nc) as tc:
        with (
            tc.tile_pool(name="consts", bufs=1) as consts,
            tc.tile_pool(name="proj", bufs=1) as proj,
            tc.tile_pool(name="stage", bufs=3) as stage,
            tc.tile_pool(name="stage_t", bufs=3) as stage_t,
            tc.tile_pool(name="p2_exp", bufs=3) as p2_exp,
            tc.tile_pool(name="p2_attn", bufs=2) as p2_attn,
            tc.tile_pool(name="p2_expt", bufs=2) as p2_expt,
            tc.tile_pool(name="p2_out", bufs=2) as p2_out,
            tc.tile_pool(name="small", bufs=8) as small,
            tc.tile_pool(name="ps_mm", bufs=3, space="PSUM") as ps_mm,
            tc.tile_pool(name="ps_pv", bufs=2, space="PSUM") as ps_pv,
            tc.tile_pool(name="ps_t", bufs=3, space="PSUM") as ps_t,
        ):
            id_bf16 = consts.tile([P, P], BF16, tag="id_bf16")
            make_identity(nc, id_bf16)

            # ---- weights: load fp32 (split per e-chunk for fast start),
            # cast bf16, PE-transpose to [d, e] ----
            wts = {}
            for name, w_d in (("wq", wq_d), ("wk", wk_d), ("wv", wv_d)):
                wT = consts.tile([P, DT, E], BF16, tag=f"{name}T")
                for et in range(ET):
                    w_f32 = stage.tile([P, D], F32, tag="wstage")
                    nc.sync.dma_start(
                        out=w_f32, in_=w_d[et * P:(et + 1) * P, :]
                    )
                    w_bf = stage.tile([P, D], BF16, tag="wbfstage")
                    nc.vector.tensor_copy(w_bf, w_f32)
                    pst = ps_t.tile([P, 4 * P], BF16, tag="ps_t")
                    for dt in range(DT):
                        nc.tensor.transpose(
                            pst[:, dt * P:(dt + 1) * P],
                            w_bf[:, dt * P:(dt + 1) * P],
                            id_bf16,
                        )
                    # pst[d_in, dt-major] -> wT[:, :, et*128...]
                    nc.vector.tensor_copy(
                        wT[:, :, et * P:(et + 1) * P],
                        pst.rearrange("p (a b) -> p a b", b=P),
                    )
                wts[name] = wT
            wqT, wkT, wvT = wts["wq"], wts["wk"], wts["wv"]

            # ---- projections ----
            # kpT/qpT resident [e_in, et, n] bf16 ; vp resident [m_in, mt, e]
            qpT = proj.tile([P, ET, N], BF16, tag="qpT")
            kpT = proj.tile([P, ET, N], BF16, tag="kpT")
            vp = proj.tile([P, MT, E], BF16, tag="vp")

            def load_and_transpose(x_d, sc):
                """Load 512 rows of x, cast bf16, PE-transpose.

                Returns xT [d_in, dt, n_local] bf16 (n_local = 512 rows of
                this super-chunk)."""
                x_f32 = stage.tile([P, 4, D], F32, tag="ldstage")
                nc.sync.dma_start(
                    out=x_f32,
                    in_=x_d[sc * 512:(sc + 1) * 512, :].rearrange(
                        "(c p) d -> p c d", p=P
                    ),
                )
                x_bf = stage.tile([P, 4, D], BF16, tag="bfstage")
                nc.vector.tensor_copy(x_bf, x_f32)
                xT = stage_t.tile([P, DT, 512], BF16, tag="xT")
                for c in range(4):
                    pst = ps_t.tile([P, 4 * P], BF16, tag="ps_t")
                    for dt in range(DT):
                        nc.tensor.transpose(
                            pst[:, dt * P:(dt + 1) * P],
                            x_bf[:, c, dt * P:(dt + 1) * P],
                            id_bf16,
                        )
                    nc.scalar.copy(
                        xT[:, :, c * P:(c + 1) * P],
                        pst.rearrange("p (a b) -> p a b", b=P),
                    )
                return xT

            # k and q projections -> T layout [e, n]
            for wT, x_d, dstT in ((wkT, k_d, kpT), (wqT, q_d, qpT)):
                for sc in range(NSC):
                    xT = load_and_transpose(x_d, sc)
                    for et in range(ET):
                        ps = ps_mm.tile([P, 512], F32, tag="ps_mm")
                        for dt in range(DT):
                            nc.tensor.matmul(
                                ps,
                                lhsT=wT[:, dt, et * P:(et + 1) * P],
                                rhs=xT[:, dt, :],
                                start=(dt == 0),
                                stop=(dt == DT - 1),
                            )
                        nc.vector.tensor_copy(
                            dstT[:, et, sc * 512:(sc + 1) * 512], ps
                        )

            # v projection -> natural layout [m, e]
            for sc in range(NSC):
                vT = load_and_transpose(v_d, sc)
                for mc in range(4):
                    mt = sc * 4 + mc
                    ps = ps_mm.tile([P, E], F32, tag="ps_mm")
                    for dt in range(DT):
                        nc.tensor.matmul(
                            ps,
                            lhsT=vT[:, dt, mc * P:(mc + 1) * P],
                            rhs=wvT[:, dt, :],
                            start=(dt == 0),
                            stop=(dt == DT - 1),
                        )
                    nc.vector.tensor_copy(vp[:, mt, :], ps)

            # ---- attention, one 128-row chunk at a time ----
            for ch in range(NCH):
                nsl = slice(ch * P, (ch + 1) * P)
                exp_bf = p2_exp.tile([P, N], BF16, tag="exp")
                rs4 = small.tile([P, 4], F32, tag="rs4")
                for mc in range(4):
                    ps = ps_mm.tile([P, 512], F32, tag="ps_mm")
                    for et in range(ET):
                        nc.tensor.matmul(
                            ps,
                            lhsT=qpT[:, et, nsl],
                            rhs=kpT[:, et, mc * 512:(mc + 1) * 512],
                            start=(et == 0),
                            stop=(et == ET - 1),
                        )
                    nc.scalar.activation(
                        out=exp_bf[:, mc * 512:(mc + 1) * 512],
                        in_=ps,
                        func=AF.Exp,
                        scale=S_EXP,
                        accum_out=rs4[:, mc:mc + 1],
                    )
                rowsum = small.tile([P, 1], F32, tag="rowsum")
                nc.vector.reduce_sum(rowsum, rs4, axis=mybir.AxisListType.X)
                rinv = small.tile([P, 1], F32, tag="rinv")
                nc.vector.reciprocal(rinv, rowsum)
                rinv_c = small.tile([P, 1], F32, tag="rinv_c")
                nc.scalar.mul(rinv_c, rinv, C_V)

                # normalized fp32 attention out to DRAM
                attn_f = p2_attn.tile([P, N], F32, tag="attn_f")
                nc.vector.tensor_scalar_mul(attn_f, in0=exp_bf, scalar1=rinv)
                nc.sync.dma_start(out=attn_d[nsl, :], in_=attn_f)

                # PE-transpose unnormalized exp for the PV matmul
                expT = p2_expt.tile([P, MT, P], BF16, tag="expT")
                for g in range(4):
                    pst = ps_t.tile([P, 4 * P], BF16, tag="ps_t")
                    for j in range(4):
                        mt = 4 * g + j
                        nc.tensor.transpose(
                            pst[:, j * P:(j + 1) * P],
                            exp_bf[:, mt * P:(mt + 1) * P],
                            id_bf16,
                        )
                    src = pst.rearrange("p (a b) -> p a b", b=P)
                    dst = expT[:, 4 * g:4 * g + 4, :]
                    if g % 2 == 0:
                        nc.scalar.copy(dst, src)
                    else:
                        nc.vector.tensor_copy(dst, src)
                pso = ps_pv.tile([P, E], F32, tag="ps_pv")
                for mt in range(MT):
                    nc.tensor.matmul(
                        pso,
                        lhsT=expT[:, mt, :],
                        rhs=vp[:, mt, :],
                        start=(mt == 0),
                        stop=(mt == MT - 1),
                    )
                out_sb = p2_out.tile([P, E], F32, tag="out_sb")
                nc.scalar.activation(
                    out=out_sb,
                    in_=pso,
                    func=AF.Identity,
                    bias=0.0,
                    scale=rinv_c[:, 0:1],
                )
                nc.sync.dma_start(out=out_d[nsl, :], in_=out_sb)

    nc.compile()
    return nc


_CACHE = {}


def _get_nc():
    if "nc" not in _CACHE:
        _CACHE["nc"] = build_nc()
    return _CACHE["nc"]


def _make_in_maps(q, k, v, Wq, Wk, Wv):
    def f32(x):
        return np.ascontiguousarray(np.asarray(x), dtype=np.float32)

    wq, wk, wv = f32(Wq), f32(Wk), f32(Wv)
    return [
        {
            "q": f32(q[i]),
            "k": f32(k[i]),
            "v": f32(v[i]),
            "Wq": wq,
            "Wk": wk,
            "Wv": wv,
        }
        for i in range(B)
    ]


def run(q, k, v, Wq, Wk, Wv, trace=False, **kwargs):
    """Run on hardware; returns ((out, (attn,)), BassKernelResults)."""
    from concourse.bass_utils import run_bass_kernel_spmd

    nc = _get_nc()
    in_maps = _make_in_maps(q, k, v, Wq, Wk, Wv)
    br = run_bass_kernel_spmd(nc, in_maps, list(range(B)), trace=trace, **kwargs)
    res = br.results
    out = np.stack([res[i]["out"] for i in range(B)])
    attn = np.stack([res[i]["attn"] for i in range(B)])
    return (out, (attn,)), br


def kernel(q, k, v, Wq, Wk, Wv):
    outputs, _ = run(q, k, v, Wq, Wk, Wv)
    return outputs
